# revision 26
# baseline (speedup 1.0000x reference)
"""AttentionPooler Trainium2 kernel: fp8e4 DoubleRow for both big matmuls.

8-core data-parallel over batch (4 batches/core), single pass over
encoder_outputs with the small weights algebraically folded on the host:
  scores[s,j] = x[s,:] @ Ac     (Ac = column-centered gamma*q~^T/8; the
                                 centering applies the LN mean exactly)
  es'[s,j] = r_s * exp(r_s*scores)        (rstd folded into exp bias/scale)
  U[j,:] = sum_s es'[s,j]*[x[s,:], mu_s, 1/r_s]   -> pooled = (U - c1)/l
  out = (pooled_h @ (gamma Wv)_h) @ Wo + beta@Wv@Wo

Both large matmuls run as fp8e4 DoubleRow (0.5 PE cycles/row, K=256 per
instruction, operand pair-strides must be 16B-aligned):
  - scores: host-uploaded pre-transposed DR-packed xT8, against ac8 + dac8
    (value + same-scale e4m3 residual at scale 2^6 -- e4m3 subnormals reach
    2^-9, so the residual is representable; this removes weight-quant error).
  - U over superchunks of 256 seq rows with residual correction:
        U = es8^T@(x8 + r8) + des8^T@x8
    where x8 = e4m3(x), r8 = e4m3(x - x8) (both uploaded), es8 = e4m3(es'),
    des8 = e4m3(es' - es8) (computed on ACT/GpSimd). The dropped des8@r8
    term is O(0.13%). 1731 PE cyc/chunk vs 2310 in fp16.
  - LayerNorm stats run on the fp8 x8 (exact in fp32 accumulation; the
    0.04%-scale stat bias is negligible); mu and 1/r ride as e4m3 columns
    768:770 of the x8 moving tile.
"""
import numpy as np
import ml_dtypes

import concourse.bass as bass
import concourse.bacc as bacc
import concourse.tile as tile
from concourse import mybir
from concourse.bass_utils import run_bass_kernel_spmd

B, S, DIM = 32, 4096, 768
H, NQ, DH = 12, 32, 64
INNER = H * DH
J = H * NQ              # 384
N_CORES = 8
B_LOC = B // N_CORES    # 4
CHUNK = 128
N_CHUNKS = S // CHUNK   # 32
ET = DIM // 128         # 6
KB = 3                  # DR k-blocks of 256 over the model dim
JT = J // 128           # 3
JH = 2
EPS = 1e-5
ACLOG2 = 6
XW = DIM + 16           # x8 moving row pitch: 768 + [mu8, invr8] + pad so the
                        # DoubleRow pair stride stays 16-byte aligned

F32 = mybir.dt.float32
F16 = mybir.dt.float16
F8 = mybir.dt.float8e4
AF = mybir.ActivationFunctionType
ALU = mybir.AluOpType
DR = mybir.MatmulPerfMode.DoubleRow
E4NP = ml_dtypes.float8_e4m3fn


def _steer_act_tables(arch: str):
    from concourse.hw_specs import get_activation_tables

    tables = get_activation_tables(arch)
    keep = "natural_log_exp_and_others"
    if keep in tables:
        for name, funcs in tables.items():
            if name != keep:
                funcs.discard(AF.Exp)


def _build_program():
    nc = bacc.Bacc(
        "TRN2", target_bir_lowering=False, debug=False, num_devices=N_CORES
    )
    _steer_act_tables(nc.m.arch)
    x8_d = nc.dram_tensor("x8", [B_LOC, S, DIM], F8, kind="ExternalInput")
    r8_d = nc.dram_tensor("r8", [B_LOC, S, DIM], F8, kind="ExternalInput")
    xt_d = nc.dram_tensor(
        "xt", [B_LOC, N_CHUNKS, 128, KB * 2 * 128], F8, kind="ExternalInput"
    )
    ac_d = nc.dram_tensor("ac", [128, KB, 2, J], F8, kind="ExternalInput")
    dac_d = nc.dram_tensor("dac", [128, KB, 2, J], F8, kind="ExternalInput")
    wv_d = nc.dram_tensor("wv", [128, ET, INNER], F16, kind="ExternalInput")
    wo_d = nc.dram_tensor("wo", [128, ET, DIM], F16, kind="ExternalInput")
    id_d = nc.dram_tensor("ident", [128, 128], F16, kind="ExternalInput")
    y_d = nc.dram_tensor("y", [B_LOC, 128, ET, NQ], F32, kind="ExternalOutput")

    with tile.TileContext(nc) as tc, \
         tc.tile_pool(name="const", bufs=1) as const, \
         tc.tile_pool(name="xin", bufs=5) as xin, \
         tc.tile_pool(name="work", bufs=12) as work, \
         tc.tile_pool(name="e8p", bufs=6) as e8p, \
         tc.tile_pool(name="stat", bufs=8) as stat, \
         tc.tile_pool(name="epi", bufs=2) as epi, \
         tc.tile_pool(name="pu", bufs=1, space="PSUM") as pu, \
         tc.tile_pool(name="pt", bufs=3, space="PSUM") as pt:

        ac_sb = const.tile([128, KB, 2, J], F8, tag="ac")
        dac_sb = const.tile([128, KB, 2, J], F8, tag="dac")
        wv_sb = const.tile([128, ET, INNER], F16, tag="wv")
        wo_sb = const.tile([128, ET, DIM], F16, tag="wo")
        eps_sb = const.tile([128, 1], F32, tag="eps")
        nc.vector.memset(eps_sb[:], EPS)
        nl2_sb = const.tile([128, 1], F32, tag="nl2")
        nc.vector.memset(nl2_sb[:], -ACLOG2 * float(np.log(2.0)))
        id_sb = const.tile([128, 128], F16, tag="ident")

        TOT = B_LOC * N_CHUNKS
        u_tiles = {}
        stage_state = {}
        ep_state = {}

        GRAN = 4                       # chunks per DMA block
        MS = GRAN // 2                 # superchunks per block
        NB = TOT // GRAN               # 32 blocks/core

        def stage_a(bi):
            b, c0 = divmod(bi * GRAN, N_CHUNKS)
            x8_t = xin.tile([128, MS, 2, XW], F8, tag="x8", name=f"x8_{bi}")
            r8_t = xin.tile([128, MS, 2, DIM], F8, tag="r8", name=f"r8_{bi}")
            xt_t = xin.tile([128, GRAN, KB, 2, 128], F8, tag="xt",
                            name=f"xt_{bi}")
            if bi == 0:
                # startup: spread DMA issue across sequencers (SP issue alone
                # costs 565ns/DMA) and order by first use -- kb0 j-halves of
                # the weights + xt[0] unblock the first matmul, x8 feeds the
                # stats chain (ACT-issued), r8 is only needed by the first
                # u_acc a few us later (DVE-issued, off the hot path).
                for w_sb, w_d in ((ac_sb, ac_d), (dac_sb, dac_d)):
                    nc.sync.dma_start(w_sb[:, 0, :, 0:192], w_d[:, 0, :, 0:192])
                for k in range(GRAN):
                    m, t = divmod(k, 2)
                    nc.sync.dma_start(
                        xt_t[:, k], xt_d[b, k].rearrange(
                            "p (kb t s) -> p kb t s", kb=KB, t=2
                        )
                    )
                    src = x8_d[b, k * 128:(k + 1) * 128, :]
                    nc.scalar.dma_start(x8_t[:, m, t, 0:DIM], src)
                    if k == 0:
                        for w_sb, w_d in ((ac_sb, ac_d), (dac_sb, dac_d)):
                            nc.sync.dma_start(w_sb[:, 0, :, 192:384],
                                              w_d[:, 0, :, 192:384])
                    elif k < KB:
                        nc.sync.dma_start(ac_sb[:, k], ac_d[:, k])
                        nc.sync.dma_start(dac_sb[:, k], dac_d[:, k])
                nc.gpsimd.dma_start(
                    r8_t[:], r8_d[b, 0:GRAN * 128].rearrange(
                        "(m t p) e -> p m t e", p=128, t=2
                    )
                )
            else:
                nc.sync.dma_start(
                    xt_t[:], xt_d[b, c0:c0 + GRAN].rearrange(
                        "c p (kb t s) -> p c kb t s", kb=KB, t=2
                    )
                )
                src = x8_d[b, c0 * 128:(c0 + GRAN) * 128, :]
                nc.sync.dma_start(
                    x8_t[:, :, :, 0:DIM],
                    src.rearrange("(m t p) e -> p m t e", p=128, t=2)
                )
                src = r8_d[b, c0 * 128:(c0 + GRAN) * 128, :]
                nc.sync.dma_start(
                    r8_t[:], src.rearrange("(m t p) e -> p m t e", p=128, t=2)
                )
            if bi == 5:
                nc.sync.dma_start(wv_sb[:], wv_d[:])
            if bi == 6:
                nc.sync.dma_start(wo_sb[:], wo_d[:])
            if bi == 4:
                nc.scalar.dma_start(id_sb[:], id_d[:])
            stage_state[("d", bi)] = (x8_t, r8_t, xt_t)

        def stage_st(bi):
            """LayerNorm row stats from the fp8 x8; mu8/invr8 land as e4m3
            columns 768:770 of the x8 moving tile."""
            x8_t, r8_t, xt_t = stage_state.pop(("d", bi))
            st = stat.tile([128, 2 * GRAN, 6], F32, tag="st", name=f"st_{bi}")

            def xg(g):
                m, t = divmod(g // 2, 2)
                return x8_t[:, m, t, (g % 2) * 384:(g % 2 + 1) * 384]

            mv = stat.tile([128, GRAN, 2], F16, tag="mv", name=f"mv_{bi}")
            lnv = stat.tile([128, GRAN], F32, tag="lnv", name=f"lnv_{bi}")
            r2_t = stat.tile([128, GRAN], F32, tag="r", name=f"r_{bi}")
            nb = stat.tile([128, GRAN], F32, tag="nb", name=f"nb_{bi}")
            spans = ([(k, k + 1) for k in range(GRAN)] if bi == 0
                     else [(0, GRAN)])
            for k0, k1 in spans:
                for g in range(2 * k0, 2 * k1):
                    nc.vector.bn_stats(st[:, g, :], xg(g))
                for k in range(k0, k1):
                    nc.vector.bn_aggr(mv[:, k, :], st[:, 2 * k:2 * k + 2, :])
                nc.scalar.activation(lnv[:, k0:k1], mv[:, k0:k1, 1], AF.Ln,
                                     bias=eps_sb[:], scale=1.0)
                nc.scalar.activation(r2_t[:, k0:k1], lnv[:, k0:k1], AF.Exp,
                                     scale=-0.5, bias=nl2_sb[:])
                nc.vector.tensor_scalar_mul(nb[:, k0:k1], lnv[:, k0:k1], -0.5)
                for k in range(k0, k1):
                    m, t = divmod(k, 2)
                    nc.scalar.activation(x8_t[:, m, t, DIM:DIM + 1],
                                         mv[:, k, 0:1], AF.Copy)
                    nc.scalar.activation(x8_t[:, m, t, DIM + 1:DIM + 2],
                                         lnv[:, k:k + 1], AF.Exp, scale=0.5)
            stage_state[bi] = (x8_t, r8_t, xt_t, r2_t, nb)

        def stage_b(bi):
            x8_t, r8_t, xt_t, r2_t, nb = stage_state.pop(bi)
            stage_state[("u", bi)] = (x8_t, r8_t)
            for k in range(GRAN):
                m, t = divmod(k, 2)
                sc = pt.tile([128, J], F32, tag="tp", name=f"sc_{bi}_{k}")
                first = True
                for kb in range(KB):
                    for w8 in (ac_sb, dac_sb):
                        for jh in range(JH):
                            nc.tensor.matmul(
                                sc[:, jh * 192:(jh + 1) * 192],
                                xt_t[:, k, kb, :, :],
                                w8[:, kb, :, jh * 192:(jh + 1) * 192],
                                start=first,
                                stop=(kb == KB - 1 and w8 is dac_sb
                                      and jh == JH - 1),
                                perf_mode=DR, skip_group_check=True,
                            )
                            first = False
                es = work.tile([128, J], F16, tag="es", name=f"es_{bi}_{k}")
                nc.scalar.activation(es[:], sc[:], AF.Exp,
                                     bias=nb[:, k:k + 1],
                                     scale=r2_t[:, k:k + 1])
                if t == 0:
                    es8 = e8p.tile([128, 2, J], F8, tag="es8",
                                   name=f"es8_{bi}_{m}")
                    des8 = e8p.tile([128, 2, J], F8, tag="des8",
                                    name=f"des8_{bi}_{m}")
                    stage_state[("e8", bi, m)] = (es8, des8)
                else:
                    es8, des8 = stage_state[("e8", bi, m)]
                # es8 = e4m3(es') on ACT; des8 = es' - es8 on GpSimd (Pool) --
                # both engines have slack, DVE is stats-bound.
                nc.scalar.activation(es8[:, t, :], es[:], AF.Copy)
                nc.gpsimd.tensor_tensor(out=des8[:, t, :], in0=es[:],
                                        in1=es8[:, t, :], op=ALU.subtract)
                if t == 1 and ("u", bi - 1) in stage_state:
                    u_acc(bi - 1, m)
            if ("u", bi - 1) in stage_state:
                stage_state.pop(("u", bi - 1))
            if (bi + 1) % NB_B == 0:
                for m in range(MS):
                    u_acc(bi, m, es_first=True)
                stage_state.pop(("u", bi))

        def u_acc(bi, m, es_first=False):
            """DR U accumulation for superchunk m (256 seq rows) of block bi:
            es8@x8 + es8@r8 + des8@x8 into the shared U PSUM banks.
            es_first=True (batch closes, where the stationaries are
            same-block-fresh) emits every es8 term before any des8 term so
            the PE overlaps the GpSimd des8 latency."""
            cw = (bi * GRAN + 2 * m) % N_CHUNKS
            b = (bi * GRAN + 2 * m) // N_CHUNKS
            x8_t, r8_t = stage_state[("u", bi)]
            es8, des8 = stage_state.pop(("e8", bi, m))
            if cw == 0:
                u_tiles[b] = (
                    [pu.tile([128, 512], F32, tag=f"u{jt}", name=f"u{jt}_{b}")
                     for jt in range(JT)],
                    pu.tile([128, 512], F32, tag="uhiA", name=f"uhiA_{b}"),
                    pu.tile([128, 512], F32, tag="uhiB", name=f"uhiB_{b}"),
                )
            ulo, uhiA, uhiB = u_tiles[b]
            first = (cw == 0)
            last = (cw == N_CHUNKS - 2)

            def esj(jt):
                return es8[:, :, jt * 128:(jt + 1) * 128]

            def desj(jt):
                return des8[:, :, jt * 128:(jt + 1) * 128]

            if es_first:
                term_passes = [((esj, x8_t), (esj, r8_t)), ((desj, x8_t),)]
            else:
                term_passes = [((esj, x8_t), (esj, r8_t), (desj, x8_t))]
            for pi, terms in enumerate(term_passes):
                fp = first and pi == 0
                for jt in range(JT):
                    for h in range(2):
                        for ti, (stf, mov) in enumerate(terms):
                            nc.tensor.matmul(
                                ulo[jt][:, h * 256:(h + 1) * 256],
                                stf(jt), mov[:, m, :, h * 256:(h + 1) * 256],
                                start=(fp and h == 0 and ti == 0), stop=last,
                                perf_mode=DR, skip_group_check=True,
                            )
                for jt in range(JT):
                    dst = (uhiA[:, jt * 256:(jt + 1) * 256] if jt < 2
                           else uhiB[:, 0:256])
                    for ti, (stf, mov) in enumerate(terms):
                        nc.tensor.matmul(
                            dst, stf(jt), mov[:, m, :, 512:768],
                            start=(fp and ti == 0 and jt != 1), stop=last,
                            perf_mode=DR, skip_group_check=True,
                        )
                for jt in range(JT):
                    # mu/invr columns against x8's cols 768:770 (the r8 term
                    # carries no mv columns)
                    for stf, mov in terms:
                        if mov is r8_t:
                            continue
                        nc.tensor.matmul(
                            uhiB[:, 256 + 2 * jt:258 + 2 * jt],
                            stf(jt), x8_t[:, m, :, DIM:DIM + 2],
                            start=False, stop=last,
                            perf_mode=DR, skip_group_check=True,
                        )

        def ep1(b):
            ulo, uhiA, uhiB = u_tiles[b]
            p2 = epi.tile([128, JT, DIM], F16, tag="p2", name=f"p2_{b}")
            for jt in range(JT):
                rl = stat.tile([128, 1], F32, tag="rl", name=f"rl_{b}_{jt}")
                nc.vector.reciprocal(rl[:], uhiB[:, 257 + 2 * jt:258 + 2 * jt])
                cc = stat.tile([128, 1], F32, tag="cc", name=f"cc_{b}_{jt}")
                nc.scalar.copy(cc[:], uhiB[:, 256 + 2 * jt:257 + 2 * jt])
                nbias = stat.tile([128, 1], F32, tag="nbias",
                                  name=f"nbias_{b}_{jt}")
                nc.vector.tensor_scalar(
                    out=nbias[:], in0=cc[:], scalar1=rl[:], scalar2=-1.0,
                    op0=ALU.mult, op1=ALU.mult,
                )
                lo_dst, lo_src = p2[:, jt, 0:512], ulo[jt][:]
                hi_dst = p2[:, jt, 512:768]
                hi_src = (uhiA[:, jt * 256:(jt + 1) * 256] if jt < 2
                          else uhiB[:, 0:256])
                if jt % 2 == 0:
                    act_pairs, dve_pairs = [(lo_dst, lo_src)], \
                        [(hi_dst, hi_src)]
                else:
                    act_pairs, dve_pairs = [(hi_dst, hi_src)], \
                        [(lo_dst, lo_src)]
                for dst, src in act_pairs:
                    nc.scalar.activation(dst, src, AF.Identity,
                                         bias=nbias[:], scale=rl[:])
                for dst, src in dve_pairs:
                    nc.vector.tensor_scalar(
                        out=dst, in0=src,
                        scalar1=cc[:], scalar2=rl[:],
                        op0=ALU.subtract, op1=ALU.mult,
                    )
            ep_state[b] = p2

        def ep2(b):
            p2 = ep_state.pop(b)
            p2T = epi.tile([128, ET, J], F16, tag="p2T", name=f"p2T_{b}")
            if b < B_LOC - 1:
                for jt in range(JT):
                    nc.sync.dma_start_transpose(
                        p2T[:, :, jt * 128:(jt + 1) * 128], p2[:, jt, :]
                    )
            else:
                for et in range(ET):
                    tp = pt.tile([128, 384], F16, tag="tp",
                                 name=f"ep_tp_{b}_{et}")
                    for jt in range(JT):
                        nc.tensor.transpose(
                            tp[:, jt * 128:(jt + 1) * 128],
                            p2[:, jt, et * 128:(et + 1) * 128],
                            id_sb[:],
                        )
                    if et % 2 == 0:
                        nc.scalar.copy(p2T[:, et, :], tp[:])
                    else:
                        nc.vector.tensor_copy(p2T[:, et, :], tp[:])
            ep_state[b] = p2T

        def ep3(b):
            p2T = ep_state.pop(b)
            ctxT = epi.tile([128, ET, NQ], F16, tag="ctxT", name=f"ctxT_{b}")
            for g in range(3):
                cp = pt.tile([128, 2, NQ], F32, tag="tp", name=f"cp_{b}_g{g}")
                for hh in range(4):
                    h = g * 4 + hh
                    dst = cp[(hh % 2) * 64:(hh % 2) * 64 + 64, hh // 2, :]
                    for et in range(ET):
                        nc.tensor.matmul(
                            dst,
                            wv_sb[:, et, h * 64:(h + 1) * 64],
                            p2T[:, et, h * NQ:(h + 1) * NQ],
                            start=(et == 0 and hh <= 1), stop=(et == ET - 1),
                            skip_group_check=True,
                        )
                if g % 2 == 0:
                    nc.scalar.copy(ctxT[:, 2 * g:2 * g + 2, :], cp[:])
                else:
                    nc.vector.tensor_copy(ctxT[:, 2 * g:2 * g + 2, :], cp[:])

            oc = epi.tile([128, ET, NQ], F32, tag="oc", name=f"oc_{b}")
            for g in range(2):
                po = pt.tile([128, 3, NQ], F32, tag="tp", name=f"po_{b}_{g}")
                for dd in range(3):
                    dt = g * 3 + dd
                    for g2 in range(ET):
                        nc.tensor.matmul(
                            po[:, dd, :],
                            wo_sb[:, g2, dt * 128:(dt + 1) * 128],
                            ctxT[:, g2, :],
                            start=(g2 == 0 and dd == 0), stop=(g2 == ET - 1),
                            skip_group_check=True,
                        )
                if g == 0:
                    nc.scalar.copy(oc[:, 0:3, :], po[:])
                else:
                    nc.vector.tensor_copy(oc[:, 3:6, :], po[:])
                # the two halves go out via different HWDGE rings (SP / ACT)
                # so their fixed issue+descriptor overheads overlap -- this
                # is the pre-drain critical chain for the last batch.
                eng = nc.sync if g == 0 else nc.scalar
                eng.dma_start(y_d[b, :, 3 * g:3 * (g + 1), :],
                              oc[:, 3 * g:3 * g + 3, :])

        NB_B = NB // B_LOC
        for bi in range(NB + 4):
            if bi < NB:
                stage_a(bi)
            if 1 <= bi < NB + 1:
                stage_st(bi - 1)
            if 2 <= bi < NB + 2:
                stage_b(bi - 2)
            for b in range(B_LOC):
                fin = (b + 1) * NB_B + 1
                if bi == fin:
                    ep1(b)
                elif bi == fin + 1:
                    ep2(b)
                elif bi == fin + 2:
                    ep3(b)

    nc.compile()
    return nc


_NC_CACHE = None


def _get_program():
    global _NC_CACHE
    if _NC_CACHE is None:
        _NC_CACHE = _build_program()
    return _NC_CACHE


def _fold_weights(queries, Wq, Wkv, Wo, gamma, beta):
    q = queries.astype(np.float64) @ Wq.astype(np.float64)
    qh = q.reshape(NQ, H, DH)
    Wk = Wkv[:, :INNER].astype(np.float64)
    Wv = Wkv[:, INNER:].astype(np.float64)
    Wk_h = Wk.reshape(DIM, H, DH)
    qt = np.einsum("nhd,ehd->hne", qh, Wk_h, optimize=True).reshape(J, DIM)
    A = (gamma.astype(np.float64)[:, None] * qt.T) / (DH ** 0.5)
    Ac = A - A.mean(axis=0, keepdims=True)
    Wvp = gamma.astype(np.float64)[:, None] * Wv
    bvwo = (beta.astype(np.float64) @ Wv) @ Wo.astype(np.float64)

    acs = Ac * float(2 ** ACLOG2)
    ac8 = acs.astype(np.float32).astype(E4NP)
    dac = (acs - ac8.astype(np.float64)).astype(np.float32).astype(E4NP)

    def dr_pack(m8):
        return np.ascontiguousarray(
            m8.reshape(KB, 2, 128, J).transpose(2, 0, 1, 3)
        )

    def tile6(m):
        return np.ascontiguousarray(
            m.reshape(ET, 128, -1).transpose(1, 0, 2)
        ).astype(np.float16)

    return (
        dr_pack(ac8),
        dr_pack(dac),
        tile6(Wvp),
        tile6(Wo.astype(np.float64)),
        bvwo.astype(np.float32),
    )


def kernel(encoder_outputs, queries, Wq, Wkv, Wo, ln_gamma, ln_beta):
    enc = np.asarray(encoder_outputs, dtype=np.float32)
    x8 = enc.astype(E4NP)
    r8 = (enc - x8.astype(np.float32)).astype(E4NP)
    xt = np.ascontiguousarray(
        x8.reshape(B, N_CHUNKS, 128, KB, 2, 128).transpose(0, 1, 5, 3, 4, 2)
    ).reshape(B, N_CHUNKS, 128, KB * 2 * 128)
    queries = np.asarray(queries, dtype=np.float32)
    Wq = np.asarray(Wq, dtype=np.float32)
    Wkv = np.asarray(Wkv, dtype=np.float32)
    Wo_np = np.asarray(Wo, dtype=np.float32)
    gamma = np.asarray(ln_gamma, dtype=np.float32)
    beta = np.asarray(ln_beta, dtype=np.float32)

    ac8, dac8, wv_t, wo_t, bvwo = _fold_weights(
        queries, Wq, Wkv, Wo_np, gamma, beta
    )

    nc = _get_program()
    in_maps = [
        {
            "x8": x8[c * B_LOC:(c + 1) * B_LOC],
            "r8": r8[c * B_LOC:(c + 1) * B_LOC],
            "xt": xt[c * B_LOC:(c + 1) * B_LOC],
            "ac": ac8,
            "dac": dac8,
            "wv": wv_t,
            "wo": wo_t,
            "ident": np.eye(128, dtype=np.float16),
        }
        for c in range(N_CORES)
    ]
    res = run_bass_kernel_spmd(nc, in_maps, list(range(N_CORES)))
    y = np.concatenate([res.results[c]["y"] for c in range(N_CORES)], axis=0)
    y = y.reshape(B, 128, ET, NQ).transpose(0, 3, 2, 1).reshape(B, NQ, DIM)
    return np.ascontiguousarray(y + bvwo[None, None, :]).astype(np.float32)


# revision 27
# speedup vs baseline: 1.0093x; 1.0093x over previous
"""AttentionPooler Trainium2 kernel: fp8e4 DoubleRow for both big matmuls.

8-core data-parallel over batch (4 batches/core), single pass over
encoder_outputs with the small weights algebraically folded on the host:
  scores[s,j] = x[s,:] @ Ac     (Ac = column-centered gamma*q~^T/8; the
                                 centering applies the LN mean exactly)
  es'[s,j] = r_s * exp(r_s*scores)        (rstd folded into exp bias/scale)
  U[j,:] = sum_s es'[s,j]*[x[s,:], mu_s, 1/r_s]   -> pooled = (U - c1)/l
  out = (pooled_h @ (gamma Wv)_h) @ Wo + beta@Wv@Wo

Both large matmuls run as fp8e4 DoubleRow (0.5 PE cycles/row, K=256 per
instruction, operand pair-strides must be 16B-aligned):
  - scores: host-uploaded pre-transposed DR-packed xT8, against ac8 + dac8
    (value + same-scale e4m3 residual at scale 2^6 -- e4m3 subnormals reach
    2^-9, so the residual is representable; this removes weight-quant error).
  - U over superchunks of 256 seq rows with residual correction:
        U = es8^T@(x8 + r8) + des8^T@x8
    where x8 = e4m3(x), r8 = e4m3(x - x8) (both uploaded), es8 = e4m3(es'),
    des8 = e4m3(es' - es8) (computed on ACT/GpSimd). The dropped des8@r8
    term is O(0.13%). 1731 PE cyc/chunk vs 2310 in fp16.
  - LayerNorm stats run on the fp8 x8 (exact in fp32 accumulation; the
    0.04%-scale stat bias is negligible); mu and 1/r ride as e4m3 columns
    768:770 of the x8 moving tile.
"""
import numpy as np
import ml_dtypes

import concourse.bass as bass
import concourse.bacc as bacc
import concourse.tile as tile
from concourse import mybir
from concourse.bass_utils import run_bass_kernel_spmd

B, S, DIM = 32, 4096, 768
H, NQ, DH = 12, 32, 64
INNER = H * DH
J = H * NQ              # 384
N_CORES = 8
B_LOC = B // N_CORES    # 4
CHUNK = 128
N_CHUNKS = S // CHUNK   # 32
ET = DIM // 128         # 6
KB = 3                  # DR k-blocks of 256 over the model dim
JT = J // 128           # 3
JH = 2
EPS = 1e-5
ACLOG2 = 6
XW = DIM + 16           # x8 moving row pitch: 768 + [mu8, invr8] + pad so the
                        # DoubleRow pair stride stays 16-byte aligned

F32 = mybir.dt.float32
F16 = mybir.dt.float16
F8 = mybir.dt.float8e4
AF = mybir.ActivationFunctionType
ALU = mybir.AluOpType
DR = mybir.MatmulPerfMode.DoubleRow
E4NP = ml_dtypes.float8_e4m3fn


def _steer_act_tables(arch: str):
    from concourse.hw_specs import get_activation_tables

    tables = get_activation_tables(arch)
    keep = "natural_log_exp_and_others"
    if keep in tables:
        for name, funcs in tables.items():
            if name != keep:
                funcs.discard(AF.Exp)


def _build_program():
    nc = bacc.Bacc(
        "TRN2", target_bir_lowering=False, debug=False, num_devices=N_CORES
    )
    _steer_act_tables(nc.m.arch)
    x8_d = nc.dram_tensor("x8", [B_LOC, S, DIM], F8, kind="ExternalInput")
    r8_d = nc.dram_tensor("r8", [B_LOC, S, DIM], F8, kind="ExternalInput")
    xt_d = nc.dram_tensor(
        "xt", [B_LOC, N_CHUNKS, 128, KB * 2 * 128], F8, kind="ExternalInput"
    )
    ac_d = nc.dram_tensor("ac", [128, KB, 2, J], F8, kind="ExternalInput")
    dac_d = nc.dram_tensor("dac", [128, KB, 2, J], F8, kind="ExternalInput")
    wv_d = nc.dram_tensor("wv", [128, ET, INNER], F16, kind="ExternalInput")
    wo_d = nc.dram_tensor("wo", [128, ET, DIM], F16, kind="ExternalInput")
    id_d = nc.dram_tensor("ident", [128, 128], F16, kind="ExternalInput")
    y_d = nc.dram_tensor("y", [B_LOC, 128, ET, NQ], F32, kind="ExternalOutput")

    with tile.TileContext(nc) as tc, \
         tc.tile_pool(name="const", bufs=1) as const, \
         tc.tile_pool(name="xin", bufs=5) as xin, \
         tc.tile_pool(name="work", bufs=12) as work, \
         tc.tile_pool(name="e8p", bufs=6) as e8p, \
         tc.tile_pool(name="stat", bufs=8) as stat, \
         tc.tile_pool(name="epi", bufs=2) as epi, \
         tc.tile_pool(name="pu", bufs=1, space="PSUM") as pu, \
         tc.tile_pool(name="pt", bufs=3, space="PSUM") as pt:

        ac_sb = const.tile([128, KB, 2, J], F8, tag="ac")
        dac_sb = const.tile([128, KB, 2, J], F8, tag="dac")
        wv_sb = const.tile([128, ET, INNER], F16, tag="wv")
        wo_sb = const.tile([128, ET, DIM], F16, tag="wo")
        eps_sb = const.tile([128, 1], F32, tag="eps")
        nc.vector.memset(eps_sb[:], EPS)
        nl2_sb = const.tile([128, 1], F32, tag="nl2")
        nc.vector.memset(nl2_sb[:], -ACLOG2 * float(np.log(2.0)))
        id_sb = const.tile([128, 128], F16, tag="ident")

        TOT = B_LOC * N_CHUNKS
        u_tiles = {}
        stage_state = {}
        ep_state = {}

        GRAN = 4                       # chunks per DMA block
        MS = GRAN // 2                 # superchunks per block
        NB = TOT // GRAN               # 32 blocks/core

        def stage_a(bi):
            b, c0 = divmod(bi * GRAN, N_CHUNKS)
            x8_t = xin.tile([128, MS, 2, XW], F8, tag="x8", name=f"x8_{bi}")
            r8_t = xin.tile([128, MS, 2, DIM], F8, tag="r8", name=f"r8_{bi}")
            xt_t = xin.tile([128, GRAN, KB, 2, 128], F8, tag="xt",
                            name=f"xt_{bi}")
            if bi == 0:
                # startup: spread DMA issue across sequencers (SP issue alone
                # costs 565ns/DMA) and order by first use -- kb0 j-halves of
                # the weights + xt[0] unblock the first matmul, x8 feeds the
                # stats chain (ACT-issued), r8 is only needed by the first
                # u_acc a few us later (DVE-issued, off the hot path).
                for w_sb, w_d in ((ac_sb, ac_d), (dac_sb, dac_d)):
                    nc.sync.dma_start(w_sb[:, 0, :, 0:192], w_d[:, 0, :, 0:192])
                for k in range(GRAN):
                    m, t = divmod(k, 2)
                    nc.sync.dma_start(
                        xt_t[:, k], xt_d[b, k].rearrange(
                            "p (kb t s) -> p kb t s", kb=KB, t=2
                        )
                    )
                    src = x8_d[b, k * 128:(k + 1) * 128, :]
                    nc.scalar.dma_start(x8_t[:, m, t, 0:DIM], src)
                    if k == 0:
                        for w_sb, w_d in ((ac_sb, ac_d), (dac_sb, dac_d)):
                            nc.sync.dma_start(w_sb[:, 0, :, 192:384],
                                              w_d[:, 0, :, 192:384])
                    elif k < KB:
                        nc.sync.dma_start(ac_sb[:, k], ac_d[:, k])
                        nc.sync.dma_start(dac_sb[:, k], dac_d[:, k])
                nc.gpsimd.dma_start(
                    r8_t[:], r8_d[b, 0:GRAN * 128].rearrange(
                        "(m t p) e -> p m t e", p=128, t=2
                    )
                )
            else:
                nc.sync.dma_start(
                    xt_t[:], xt_d[b, c0:c0 + GRAN].rearrange(
                        "c p (kb t s) -> p c kb t s", kb=KB, t=2
                    )
                )
                src = x8_d[b, c0 * 128:(c0 + GRAN) * 128, :]
                nc.sync.dma_start(
                    x8_t[:, :, :, 0:DIM],
                    src.rearrange("(m t p) e -> p m t e", p=128, t=2)
                )
                src = r8_d[b, c0 * 128:(c0 + GRAN) * 128, :]
                nc.sync.dma_start(
                    r8_t[:], src.rearrange("(m t p) e -> p m t e", p=128, t=2)
                )
            if bi == 5:
                nc.sync.dma_start(wv_sb[:], wv_d[:])
            if bi == 6:
                nc.sync.dma_start(wo_sb[:], wo_d[:])
            if bi == 4:
                nc.scalar.dma_start(id_sb[:], id_d[:])
            stage_state[("d", bi)] = (x8_t, r8_t, xt_t)

        def stage_st(bi):
            """LayerNorm row stats from the fp8 x8; mu8/invr8 land as e4m3
            columns 768:770 of the x8 moving tile."""
            x8_t, r8_t, xt_t = stage_state.pop(("d", bi))
            st = stat.tile([128, 2 * GRAN, 6], F32, tag="st", name=f"st_{bi}")

            def xg(g):
                m, t = divmod(g // 2, 2)
                return x8_t[:, m, t, (g % 2) * 384:(g % 2 + 1) * 384]

            mv = stat.tile([128, GRAN, 2], F16, tag="mv", name=f"mv_{bi}")
            lnv = stat.tile([128, GRAN], F32, tag="lnv", name=f"lnv_{bi}")
            r2_t = stat.tile([128, GRAN], F32, tag="r", name=f"r_{bi}")
            nb = stat.tile([128, GRAN], F32, tag="nb", name=f"nb_{bi}")
            spans = ([(k, k + 1) for k in range(GRAN)] if bi == 0
                     else [(0, GRAN)])
            for k0, k1 in spans:
                for g in range(2 * k0, 2 * k1):
                    nc.vector.bn_stats(st[:, g, :], xg(g))
                for k in range(k0, k1):
                    nc.vector.bn_aggr(mv[:, k, :], st[:, 2 * k:2 * k + 2, :])
                nc.scalar.activation(lnv[:, k0:k1], mv[:, k0:k1, 1], AF.Ln,
                                     bias=eps_sb[:], scale=1.0)
                nc.scalar.activation(r2_t[:, k0:k1], lnv[:, k0:k1], AF.Exp,
                                     scale=-0.5, bias=nl2_sb[:])
                nc.vector.tensor_scalar_mul(nb[:, k0:k1], lnv[:, k0:k1], -0.5)
                for k in range(k0, k1):
                    m, t = divmod(k, 2)
                    nc.scalar.activation(x8_t[:, m, t, DIM:DIM + 1],
                                         mv[:, k, 0:1], AF.Copy)
                    nc.scalar.activation(x8_t[:, m, t, DIM + 1:DIM + 2],
                                         lnv[:, k:k + 1], AF.Exp, scale=0.5)
            stage_state[bi] = (x8_t, r8_t, xt_t, r2_t, nb)

        def stage_b(bi):
            x8_t, r8_t, xt_t, r2_t, nb = stage_state.pop(bi)
            stage_state[("u", bi)] = (x8_t, r8_t)
            for k in range(GRAN):
                m, t = divmod(k, 2)
                sc = pt.tile([128, J], F32, tag="tp", name=f"sc_{bi}_{k}")
                first = True
                for kb in range(KB):
                    for w8 in (ac_sb, dac_sb):
                        for jh in range(JH):
                            nc.tensor.matmul(
                                sc[:, jh * 192:(jh + 1) * 192],
                                xt_t[:, k, kb, :, :],
                                w8[:, kb, :, jh * 192:(jh + 1) * 192],
                                start=first,
                                stop=(kb == KB - 1 and w8 is dac_sb
                                      and jh == JH - 1),
                                perf_mode=DR, skip_group_check=True,
                            )
                            first = False
                es = work.tile([128, J], F16, tag="es", name=f"es_{bi}_{k}")
                nc.scalar.activation(es[:], sc[:], AF.Exp,
                                     bias=nb[:, k:k + 1],
                                     scale=r2_t[:, k:k + 1])
                if t == 0:
                    es8 = e8p.tile([128, 2, J], F8, tag="es8",
                                   name=f"es8_{bi}_{m}")
                    des8 = e8p.tile([128, 2, J], F8, tag="des8",
                                    name=f"des8_{bi}_{m}")
                    stage_state[("e8", bi, m)] = (es8, des8)
                else:
                    es8, des8 = stage_state[("e8", bi, m)]
                # es8 = e4m3(es') on ACT; des8 = es' - es8 on GpSimd (Pool) --
                # both engines have slack, DVE is stats-bound.
                nc.scalar.activation(es8[:, t, :], es[:], AF.Copy)
                nc.gpsimd.tensor_tensor(out=des8[:, t, :], in0=es[:],
                                        in1=es8[:, t, :], op=ALU.subtract)
                if t == 1 and ("u", bi - 1) in stage_state:
                    u_acc(bi - 1, m)
            if ("u", bi - 1) in stage_state:
                stage_state.pop(("u", bi - 1))
            if (bi + 1) % NB_B == 0:
                for m in range(MS):
                    u_acc(bi, m, es_first=True)
                stage_state.pop(("u", bi))

        def u_acc(bi, m, es_first=False):
            """DR U accumulation for superchunk m (256 seq rows) of block bi:
            es8@x8 + es8@r8 + des8@x8 into the shared U PSUM banks.
            es_first=True (batch closes, where the stationaries are
            same-block-fresh) emits every es8 term before any des8 term so
            the PE overlaps the GpSimd des8 latency."""
            cw = (bi * GRAN + 2 * m) % N_CHUNKS
            b = (bi * GRAN + 2 * m) // N_CHUNKS
            x8_t, r8_t = stage_state[("u", bi)]
            es8, des8 = stage_state.pop(("e8", bi, m))
            if cw == 0:
                u_tiles[b] = (
                    [pu.tile([128, 512], F32, tag=f"u{jt}", name=f"u{jt}_{b}")
                     for jt in range(JT)],
                    pu.tile([128, 512], F32, tag="uhiA", name=f"uhiA_{b}"),
                    pu.tile([128, 512], F32, tag="uhiB", name=f"uhiB_{b}"),
                )
            ulo, uhiA, uhiB = u_tiles[b]
            first = (cw == 0)
            last = (cw == N_CHUNKS - 2)

            def esj(jt):
                return es8[:, :, jt * 128:(jt + 1) * 128]

            def desj(jt):
                return des8[:, :, jt * 128:(jt + 1) * 128]

            if es_first:
                term_passes = [((esj, x8_t), (esj, r8_t)), ((desj, x8_t),)]
            else:
                term_passes = [((esj, x8_t), (esj, r8_t), (desj, x8_t))]
            for pi, terms in enumerate(term_passes):
                fp = first and pi == 0
                for jt in range(JT):
                    for h in range(2):
                        for ti, (stf, mov) in enumerate(terms):
                            nc.tensor.matmul(
                                ulo[jt][:, h * 256:(h + 1) * 256],
                                stf(jt), mov[:, m, :, h * 256:(h + 1) * 256],
                                start=(fp and h == 0 and ti == 0), stop=last,
                                perf_mode=DR, skip_group_check=True,
                            )
                for jt in range(JT):
                    dst = (uhiA[:, jt * 256:(jt + 1) * 256] if jt < 2
                           else uhiB[:, 0:256])
                    for ti, (stf, mov) in enumerate(terms):
                        nc.tensor.matmul(
                            dst, stf(jt), mov[:, m, :, 512:768],
                            start=(fp and ti == 0 and jt != 1), stop=last,
                            perf_mode=DR, skip_group_check=True,
                        )
                for jt in range(JT):
                    # mu/invr columns against x8's cols 768:770 (the r8 term
                    # carries no mv columns)
                    for stf, mov in terms:
                        if mov is r8_t:
                            continue
                        nc.tensor.matmul(
                            uhiB[:, 256 + 2 * jt:258 + 2 * jt],
                            stf(jt), x8_t[:, m, :, DIM:DIM + 2],
                            start=False, stop=last,
                            perf_mode=DR, skip_group_check=True,
                        )

        def ep1(b):
            ulo, uhiA, uhiB = u_tiles[b]
            p2 = epi.tile([128, JT, DIM], F16, tag="p2", name=f"p2_{b}")
            for jt in range(JT):
                rl = stat.tile([128, 1], F32, tag="rl", name=f"rl_{b}_{jt}")
                nc.vector.reciprocal(rl[:], uhiB[:, 257 + 2 * jt:258 + 2 * jt])
                cc = stat.tile([128, 1], F32, tag="cc", name=f"cc_{b}_{jt}")
                nc.scalar.copy(cc[:], uhiB[:, 256 + 2 * jt:257 + 2 * jt])
                nbias = stat.tile([128, 1], F32, tag="nbias",
                                  name=f"nbias_{b}_{jt}")
                nc.vector.tensor_scalar(
                    out=nbias[:], in0=cc[:], scalar1=rl[:], scalar2=-1.0,
                    op0=ALU.mult, op1=ALU.mult,
                )
                lo_dst, lo_src = p2[:, jt, 0:512], ulo[jt][:]
                hi_dst = p2[:, jt, 512:768]
                hi_src = (uhiA[:, jt * 256:(jt + 1) * 256] if jt < 2
                          else uhiB[:, 0:256])
                if jt % 2 == 0:
                    act_pairs, dve_pairs = [(lo_dst, lo_src)], \
                        [(hi_dst, hi_src)]
                else:
                    act_pairs, dve_pairs = [(hi_dst, hi_src)], \
                        [(lo_dst, lo_src)]
                for dst, src in act_pairs:
                    nc.scalar.activation(dst, src, AF.Identity,
                                         bias=nbias[:], scale=rl[:])
                for dst, src in dve_pairs:
                    nc.vector.tensor_scalar(
                        out=dst, in0=src,
                        scalar1=cc[:], scalar2=rl[:],
                        op0=ALU.subtract, op1=ALU.mult,
                    )
            ep_state[b] = p2

        def ep2(b):
            p2 = ep_state.pop(b)
            p2T = epi.tile([128, ET, J], F16, tag="p2T", name=f"p2T_{b}")
            if b < B_LOC - 1:
                for jt in range(JT):
                    nc.sync.dma_start_transpose(
                        p2T[:, :, jt * 128:(jt + 1) * 128], p2[:, jt, :]
                    )
            else:
                for et in range(ET):
                    tp = pt.tile([128, 384], F16, tag="tp",
                                 name=f"ep_tp_{b}_{et}")
                    for jt in range(JT):
                        nc.tensor.transpose(
                            tp[:, jt * 128:(jt + 1) * 128],
                            p2[:, jt, et * 128:(et + 1) * 128],
                            id_sb[:],
                        )
                    if et % 2 == 0:
                        nc.scalar.copy(p2T[:, et, :], tp[:])
                    else:
                        nc.vector.tensor_copy(p2T[:, et, :], tp[:])
            ep_state[b] = p2T

        def ep3(b):
            p2T = ep_state.pop(b)
            ctxT = epi.tile([128, ET, NQ], F16, tag="ctxT", name=f"ctxT_{b}")
            for g in range(3):
                cp = pt.tile([128, 2, NQ], F32, tag="tp", name=f"cp_{b}_g{g}")
                for hh in range(4):
                    h = g * 4 + hh
                    dst = cp[(hh % 2) * 64:(hh % 2) * 64 + 64, hh // 2, :]
                    for et in range(ET):
                        nc.tensor.matmul(
                            dst,
                            wv_sb[:, et, h * 64:(h + 1) * 64],
                            p2T[:, et, h * NQ:(h + 1) * NQ],
                            start=(et == 0 and hh <= 1), stop=(et == ET - 1),
                            skip_group_check=True,
                        )
                if g % 2 == 0:
                    nc.scalar.copy(ctxT[:, 2 * g:2 * g + 2, :], cp[:])
                else:
                    nc.vector.tensor_copy(ctxT[:, 2 * g:2 * g + 2, :], cp[:])

            oc = epi.tile([128, ET, NQ], F32, tag="oc", name=f"oc_{b}")
            for g in range(2):
                po = pt.tile([128, 3, NQ], F32, tag="tp", name=f"po_{b}_{g}")
                for dd in range(3):
                    dt = g * 3 + dd
                    for g2 in range(ET):
                        nc.tensor.matmul(
                            po[:, dd, :],
                            wo_sb[:, g2, dt * 128:(dt + 1) * 128],
                            ctxT[:, g2, :],
                            start=(g2 == 0 and dd == 0), stop=(g2 == ET - 1),
                            skip_group_check=True,
                        )
                if g == 0:
                    nc.scalar.copy(oc[:, 0:3, :], po[:])
                else:
                    nc.vector.tensor_copy(oc[:, 3:6, :], po[:])
                nc.sync.dma_start(y_d[b, :, 3 * g:3 * (g + 1), :],
                                  oc[:, 3 * g:3 * g + 3, :])

        NB_B = NB // B_LOC
        for bi in range(NB + 4):
            if bi < NB:
                stage_a(bi)
            if 1 <= bi < NB + 1:
                stage_st(bi - 1)
            if 2 <= bi < NB + 2:
                stage_b(bi - 2)
            for b in range(B_LOC):
                fin = (b + 1) * NB_B + 1
                if bi == fin:
                    ep1(b)
                elif bi == fin + 1:
                    ep2(b)
                elif bi == fin + 2:
                    ep3(b)

    nc.compile()
    return nc


_NC_CACHE = None


def _get_program():
    global _NC_CACHE
    if _NC_CACHE is None:
        _NC_CACHE = _build_program()
    return _NC_CACHE


def _fold_weights(queries, Wq, Wkv, Wo, gamma, beta):
    q = queries.astype(np.float64) @ Wq.astype(np.float64)
    qh = q.reshape(NQ, H, DH)
    Wk = Wkv[:, :INNER].astype(np.float64)
    Wv = Wkv[:, INNER:].astype(np.float64)
    Wk_h = Wk.reshape(DIM, H, DH)
    qt = np.einsum("nhd,ehd->hne", qh, Wk_h, optimize=True).reshape(J, DIM)
    A = (gamma.astype(np.float64)[:, None] * qt.T) / (DH ** 0.5)
    Ac = A - A.mean(axis=0, keepdims=True)
    Wvp = gamma.astype(np.float64)[:, None] * Wv
    bvwo = (beta.astype(np.float64) @ Wv) @ Wo.astype(np.float64)

    acs = Ac * float(2 ** ACLOG2)
    ac8 = acs.astype(np.float32).astype(E4NP)
    dac = (acs - ac8.astype(np.float64)).astype(np.float32).astype(E4NP)

    def dr_pack(m8):
        return np.ascontiguousarray(
            m8.reshape(KB, 2, 128, J).transpose(2, 0, 1, 3)
        )

    def tile6(m):
        return np.ascontiguousarray(
            m.reshape(ET, 128, -1).transpose(1, 0, 2)
        ).astype(np.float16)

    return (
        dr_pack(ac8),
        dr_pack(dac),
        tile6(Wvp),
        tile6(Wo.astype(np.float64)),
        bvwo.astype(np.float32),
    )


def kernel(encoder_outputs, queries, Wq, Wkv, Wo, ln_gamma, ln_beta):
    enc = np.asarray(encoder_outputs, dtype=np.float32)
    x8 = enc.astype(E4NP)
    r8 = (enc - x8.astype(np.float32)).astype(E4NP)
    xt = np.ascontiguousarray(
        x8.reshape(B, N_CHUNKS, 128, KB, 2, 128).transpose(0, 1, 5, 3, 4, 2)
    ).reshape(B, N_CHUNKS, 128, KB * 2 * 128)
    queries = np.asarray(queries, dtype=np.float32)
    Wq = np.asarray(Wq, dtype=np.float32)
    Wkv = np.asarray(Wkv, dtype=np.float32)
    Wo_np = np.asarray(Wo, dtype=np.float32)
    gamma = np.asarray(ln_gamma, dtype=np.float32)
    beta = np.asarray(ln_beta, dtype=np.float32)

    ac8, dac8, wv_t, wo_t, bvwo = _fold_weights(
        queries, Wq, Wkv, Wo_np, gamma, beta
    )

    nc = _get_program()
    in_maps = [
        {
            "x8": x8[c * B_LOC:(c + 1) * B_LOC],
            "r8": r8[c * B_LOC:(c + 1) * B_LOC],
            "xt": xt[c * B_LOC:(c + 1) * B_LOC],
            "ac": ac8,
            "dac": dac8,
            "wv": wv_t,
            "wo": wo_t,
            "ident": np.eye(128, dtype=np.float16),
        }
        for c in range(N_CORES)
    ]
    res = run_bass_kernel_spmd(nc, in_maps, list(range(N_CORES)))
    y = np.concatenate([res.results[c]["y"] for c in range(N_CORES)], axis=0)
    y = y.reshape(B, 128, ET, NQ).transpose(0, 3, 2, 1).reshape(B, NQ, DIM)
    return np.ascontiguousarray(y + bvwo[None, None, :]).astype(np.float32)


# revision 29
# speedup vs baseline: 1.0189x; 1.0095x over previous
"""AttentionPooler Trainium2 kernel: fp8e4 DoubleRow for both big matmuls.

8-core data-parallel over batch (4 batches/core), single pass over
encoder_outputs with the small weights algebraically folded on the host:
  scores[s,j] = x[s,:] @ Ac     (Ac = column-centered gamma*q~^T/8; the
                                 centering applies the LN mean exactly)
  es'[s,j] = r_s * exp(r_s*scores)        (rstd folded into exp bias/scale)
  U[j,:] = sum_s es'[s,j]*[x[s,:], mu_s, 1/r_s]   -> pooled = (U - c1)/l
  out = (pooled_h @ (gamma Wv)_h) @ Wo + beta@Wv@Wo

Both large matmuls run as fp8e4 DoubleRow (0.5 PE cycles/row, K=256 per
instruction, operand pair-strides must be 16B-aligned):
  - scores: host-uploaded pre-transposed DR-packed xT8, against ac8 + dac8
    (value + same-scale e4m3 residual at scale 2^6 -- e4m3 subnormals reach
    2^-9, so the residual is representable; this removes weight-quant error).
  - U over superchunks of 256 seq rows with residual correction:
        U = es8^T@(x8 + r8) + des8^T@x8
    where x8 = e4m3(x), r8 = e4m3(x - x8) (both uploaded), es8 = e4m3(es'),
    des8 = e4m3(es' - es8) (computed on ACT/GpSimd). The dropped des8@r8
    term is O(0.13%). 1731 PE cyc/chunk vs 2310 in fp16.
  - LayerNorm stats run on the fp8 x8 (exact in fp32 accumulation; the
    0.04%-scale stat bias is negligible); mu and 1/r ride as e4m3 columns
    768:770 of the x8 moving tile.
"""
import numpy as np
import ml_dtypes

import concourse.bass as bass
import concourse.bacc as bacc
import concourse.tile as tile
from concourse import mybir
from concourse.bass_utils import run_bass_kernel_spmd

B, S, DIM = 32, 4096, 768
H, NQ, DH = 12, 32, 64
INNER = H * DH
J = H * NQ              # 384
N_CORES = 8
B_LOC = B // N_CORES    # 4
CHUNK = 128
N_CHUNKS = S // CHUNK   # 32
ET = DIM // 128         # 6
KB = 3                  # DR k-blocks of 256 over the model dim
JT = J // 128           # 3
JH = 2
EPS = 1e-5
ACLOG2 = 6
XW = DIM + 16           # x8 moving row pitch: 768 + [mu8, invr8] + pad so the
                        # DoubleRow pair stride stays 16-byte aligned

F32 = mybir.dt.float32
F16 = mybir.dt.float16
F8 = mybir.dt.float8e4
AF = mybir.ActivationFunctionType
ALU = mybir.AluOpType
DR = mybir.MatmulPerfMode.DoubleRow
E4NP = ml_dtypes.float8_e4m3fn


def _steer_act_tables(arch: str):
    from concourse.hw_specs import get_activation_tables

    tables = get_activation_tables(arch)
    keep = "natural_log_exp_and_others"
    if keep in tables:
        for name, funcs in tables.items():
            if name != keep:
                funcs.discard(AF.Exp)


def _build_program():
    nc = bacc.Bacc(
        "TRN2", target_bir_lowering=False, debug=False, num_devices=N_CORES
    )
    _steer_act_tables(nc.m.arch)
    x8_d = nc.dram_tensor("x8", [B_LOC, S, DIM], F8, kind="ExternalInput")
    r8_d = nc.dram_tensor("r8", [B_LOC, S, DIM], F8, kind="ExternalInput")
    xt_d = nc.dram_tensor(
        "xt", [B_LOC, N_CHUNKS, 128, KB * 2 * 128], F8, kind="ExternalInput"
    )
    ac_d = nc.dram_tensor("ac", [128, KB, 2, J], F8, kind="ExternalInput")
    dac_d = nc.dram_tensor("dac", [128, KB, 2, J], F8, kind="ExternalInput")
    wv_d = nc.dram_tensor("wv", [128, ET, INNER], F16, kind="ExternalInput")
    wo_d = nc.dram_tensor("wo", [128, ET, DIM], F16, kind="ExternalInput")
    id_d = nc.dram_tensor("ident", [128, 128], F16, kind="ExternalInput")
    y_d = nc.dram_tensor("y", [B_LOC, 128, ET, NQ], F32, kind="ExternalOutput")

    with tile.TileContext(nc) as tc, \
         tc.tile_pool(name="const", bufs=1) as const, \
         tc.tile_pool(name="xin", bufs=5) as xin, \
         tc.tile_pool(name="work", bufs=12) as work, \
         tc.tile_pool(name="e8p", bufs=6) as e8p, \
         tc.tile_pool(name="stat", bufs=8) as stat, \
         tc.tile_pool(name="epi", bufs=2) as epi, \
         tc.tile_pool(name="pu", bufs=1, space="PSUM") as pu, \
         tc.tile_pool(name="pt", bufs=3, space="PSUM") as pt:

        ac_sb = const.tile([128, KB, 2, J], F8, tag="ac")
        dac_sb = const.tile([128, KB, 2, J], F8, tag="dac")
        wv_sb = const.tile([128, ET, INNER], F16, tag="wv")
        wo_sb = const.tile([128, ET, DIM], F16, tag="wo")
        eps_sb = const.tile([128, 1], F32, tag="eps")
        nc.vector.memset(eps_sb[:], EPS)
        nl2_sb = const.tile([128, 1], F32, tag="nl2")
        nc.vector.memset(nl2_sb[:], -ACLOG2 * float(np.log(2.0)))
        id_sb = const.tile([128, 128], F16, tag="ident")

        TOT = B_LOC * N_CHUNKS
        u_tiles = {}
        stage_state = {}
        ep_state = {}

        GRAN = 4                       # chunks per DMA block
        MS = GRAN // 2                 # superchunks per block
        NB = TOT // GRAN               # 32 blocks/core

        def stage_a(bi):
            b, c0 = divmod(bi * GRAN, N_CHUNKS)
            x8_t = xin.tile([128, MS, 2, XW], F8, tag="x8", name=f"x8_{bi}")
            r8_t = xin.tile([128, MS, 2, DIM], F8, tag="r8", name=f"r8_{bi}")
            xt_t = xin.tile([128, GRAN, KB, 2, 128], F8, tag="xt",
                            name=f"xt_{bi}")
            if bi == 0:
                # startup: spread DMA issue across sequencers (SP issue alone
                # costs 565ns/DMA) and order by first use -- kb0 j-halves of
                # the weights + xt[0] unblock the first matmul, x8 feeds the
                # stats chain (ACT-issued), r8 is only needed by the first
                # u_acc a few us later (DVE-issued, off the hot path).
                nc.sync.dma_start(ac_sb[:, 0, :, 0:192], ac_d[:, 0, :, 0:192])
                for k in range(GRAN):
                    m, t = divmod(k, 2)
                    nc.sync.dma_start(
                        xt_t[:, k], xt_d[b, k].rearrange(
                            "p (kb t s) -> p kb t s", kb=KB, t=2
                        )
                    )
                    src = x8_d[b, k * 128:(k + 1) * 128, :]
                    nc.scalar.dma_start(x8_t[:, m, t, 0:DIM], src)
                    if k == 0:
                        nc.sync.dma_start(ac_sb[:, 0, :, 192:384],
                                          ac_d[:, 0, :, 192:384])
                    elif k < KB:
                        nc.sync.dma_start(ac_sb[:, k], ac_d[:, k])
                nc.gpsimd.dma_start(
                    r8_t[:], r8_d[b, 0:GRAN * 128].rearrange(
                        "(m t p) e -> p m t e", p=128, t=2
                    )
                )
                # block 0 skips the dac residual terms, so dac loads behind
                # all of block 0's data (needed from block 1, ~2 blocks away)
                nc.sync.dma_start(dac_sb[:], dac_d[:])
            else:
                nc.sync.dma_start(
                    xt_t[:], xt_d[b, c0:c0 + GRAN].rearrange(
                        "c p (kb t s) -> p c kb t s", kb=KB, t=2
                    )
                )
                src = x8_d[b, c0 * 128:(c0 + GRAN) * 128, :]
                nc.sync.dma_start(
                    x8_t[:, :, :, 0:DIM],
                    src.rearrange("(m t p) e -> p m t e", p=128, t=2)
                )
                src = r8_d[b, c0 * 128:(c0 + GRAN) * 128, :]
                nc.sync.dma_start(
                    r8_t[:], src.rearrange("(m t p) e -> p m t e", p=128, t=2)
                )
            if bi == 5:
                nc.sync.dma_start(wv_sb[:], wv_d[:])
            if bi == 6:
                nc.sync.dma_start(wo_sb[:], wo_d[:])
            if bi == 4:
                nc.scalar.dma_start(id_sb[:], id_d[:])
            stage_state[("d", bi)] = (x8_t, r8_t, xt_t)

        def stage_st(bi):
            """LayerNorm row stats from the fp8 x8; mu8/invr8 land as e4m3
            columns 768:770 of the x8 moving tile."""
            x8_t, r8_t, xt_t = stage_state.pop(("d", bi))
            st = stat.tile([128, 2 * GRAN, 6], F32, tag="st", name=f"st_{bi}")

            def xg(g):
                m, t = divmod(g // 2, 2)
                return x8_t[:, m, t, (g % 2) * 384:(g % 2 + 1) * 384]

            mv = stat.tile([128, GRAN, 2], F16, tag="mv", name=f"mv_{bi}")
            lnv = stat.tile([128, GRAN], F32, tag="lnv", name=f"lnv_{bi}")
            r2_t = stat.tile([128, GRAN], F32, tag="r", name=f"r_{bi}")
            nb = stat.tile([128, GRAN], F32, tag="nb", name=f"nb_{bi}")
            spans = ([(k, k + 1) for k in range(GRAN)] if bi == 0
                     else [(0, GRAN)])
            for k0, k1 in spans:
                for g in range(2 * k0, 2 * k1):
                    nc.vector.bn_stats(st[:, g, :], xg(g))
                for k in range(k0, k1):
                    nc.vector.bn_aggr(mv[:, k, :], st[:, 2 * k:2 * k + 2, :])
                nc.scalar.activation(lnv[:, k0:k1], mv[:, k0:k1, 1], AF.Ln,
                                     bias=eps_sb[:], scale=1.0)
                nc.scalar.activation(r2_t[:, k0:k1], lnv[:, k0:k1], AF.Exp,
                                     scale=-0.5, bias=nl2_sb[:])
                nc.vector.tensor_scalar_mul(nb[:, k0:k1], lnv[:, k0:k1], -0.5)
                for k in range(k0, k1):
                    m, t = divmod(k, 2)
                    nc.scalar.activation(x8_t[:, m, t, DIM:DIM + 1],
                                         mv[:, k, 0:1], AF.Copy)
                    nc.scalar.activation(x8_t[:, m, t, DIM + 1:DIM + 2],
                                         lnv[:, k:k + 1], AF.Exp, scale=0.5)
            stage_state[bi] = (x8_t, r8_t, xt_t, r2_t, nb)

        def stage_b(bi):
            x8_t, r8_t, xt_t, r2_t, nb = stage_state.pop(bi)
            stage_state[("u", bi)] = (x8_t, r8_t)
            for k in range(GRAN):
                m, t = divmod(k, 2)
                sc = pt.tile([128, J], F32, tag="tp", name=f"sc_{bi}_{k}")
                # block 0 drops the dac residual (-0.8us of startup-critical
                # DMA; +0.2% global error, still 2x under the gate)
                w8s = (ac_sb,) if bi == 0 else (ac_sb, dac_sb)
                first = True
                for kb in range(KB):
                    for w8 in w8s:
                        for jh in range(JH):
                            nc.tensor.matmul(
                                sc[:, jh * 192:(jh + 1) * 192],
                                xt_t[:, k, kb, :, :],
                                w8[:, kb, :, jh * 192:(jh + 1) * 192],
                                start=first,
                                stop=(kb == KB - 1 and w8 is w8s[-1]
                                      and jh == JH - 1),
                                perf_mode=DR, skip_group_check=True,
                            )
                            first = False
                es = work.tile([128, J], F16, tag="es", name=f"es_{bi}_{k}")
                nc.scalar.activation(es[:], sc[:], AF.Exp,
                                     bias=nb[:, k:k + 1],
                                     scale=r2_t[:, k:k + 1])
                if t == 0:
                    es8 = e8p.tile([128, 2, J], F8, tag="es8",
                                   name=f"es8_{bi}_{m}")
                    des8 = e8p.tile([128, 2, J], F8, tag="des8",
                                    name=f"des8_{bi}_{m}")
                    stage_state[("e8", bi, m)] = (es8, des8)
                else:
                    es8, des8 = stage_state[("e8", bi, m)]
                # es8 = e4m3(es') on ACT; des8 = es' - es8 on GpSimd (Pool) --
                # both engines have slack, DVE is stats-bound.
                nc.scalar.activation(es8[:, t, :], es[:], AF.Copy)
                nc.gpsimd.tensor_tensor(out=des8[:, t, :], in0=es[:],
                                        in1=es8[:, t, :], op=ALU.subtract)
                if t == 1 and ("u", bi - 1) in stage_state:
                    u_acc(bi - 1, m)
            if ("u", bi - 1) in stage_state:
                stage_state.pop(("u", bi - 1))
            if (bi + 1) % NB_B == 0:
                for m in range(MS):
                    u_acc(bi, m, es_first=True)
                stage_state.pop(("u", bi))

        def u_acc(bi, m, es_first=False):
            """DR U accumulation for superchunk m (256 seq rows) of block bi:
            es8@x8 + es8@r8 + des8@x8 into the shared U PSUM banks.
            es_first=True (batch closes, where the stationaries are
            same-block-fresh) emits every es8 term before any des8 term so
            the PE overlaps the GpSimd des8 latency."""
            cw = (bi * GRAN + 2 * m) % N_CHUNKS
            b = (bi * GRAN + 2 * m) // N_CHUNKS
            x8_t, r8_t = stage_state[("u", bi)]
            es8, des8 = stage_state.pop(("e8", bi, m))
            if cw == 0:
                u_tiles[b] = (
                    [pu.tile([128, 512], F32, tag=f"u{jt}", name=f"u{jt}_{b}")
                     for jt in range(JT)],
                    pu.tile([128, 512], F32, tag="uhiA", name=f"uhiA_{b}"),
                    pu.tile([128, 512], F32, tag="uhiB", name=f"uhiB_{b}"),
                )
            ulo, uhiA, uhiB = u_tiles[b]
            first = (cw == 0)
            last = (cw == N_CHUNKS - 2)

            def esj(jt):
                return es8[:, :, jt * 128:(jt + 1) * 128]

            def desj(jt):
                return des8[:, :, jt * 128:(jt + 1) * 128]

            if es_first:
                term_passes = [((esj, x8_t), (esj, r8_t)), ((desj, x8_t),)]
            else:
                term_passes = [((esj, x8_t), (esj, r8_t), (desj, x8_t))]
            for pi, terms in enumerate(term_passes):
                fp = first and pi == 0
                for jt in range(JT):
                    for h in range(2):
                        for ti, (stf, mov) in enumerate(terms):
                            nc.tensor.matmul(
                                ulo[jt][:, h * 256:(h + 1) * 256],
                                stf(jt), mov[:, m, :, h * 256:(h + 1) * 256],
                                start=(fp and h == 0 and ti == 0), stop=last,
                                perf_mode=DR, skip_group_check=True,
                            )
                for jt in range(JT):
                    dst = (uhiA[:, jt * 256:(jt + 1) * 256] if jt < 2
                           else uhiB[:, 0:256])
                    for ti, (stf, mov) in enumerate(terms):
                        nc.tensor.matmul(
                            dst, stf(jt), mov[:, m, :, 512:768],
                            start=(fp and ti == 0 and jt != 1), stop=last,
                            perf_mode=DR, skip_group_check=True,
                        )
                for jt in range(JT):
                    # mu/invr columns against x8's cols 768:770 (the r8 term
                    # carries no mv columns)
                    for stf, mov in terms:
                        if mov is r8_t:
                            continue
                        nc.tensor.matmul(
                            uhiB[:, 256 + 2 * jt:258 + 2 * jt],
                            stf(jt), x8_t[:, m, :, DIM:DIM + 2],
                            start=False, stop=last,
                            perf_mode=DR, skip_group_check=True,
                        )

        def ep1(b):
            ulo, uhiA, uhiB = u_tiles[b]
            p2 = epi.tile([128, JT, DIM], F16, tag="p2", name=f"p2_{b}")
            for jt in range(JT):
                rl = stat.tile([128, 1], F32, tag="rl", name=f"rl_{b}_{jt}")
                nc.vector.reciprocal(rl[:], uhiB[:, 257 + 2 * jt:258 + 2 * jt])
                cc = stat.tile([128, 1], F32, tag="cc", name=f"cc_{b}_{jt}")
                nc.scalar.copy(cc[:], uhiB[:, 256 + 2 * jt:257 + 2 * jt])
                nbias = stat.tile([128, 1], F32, tag="nbias",
                                  name=f"nbias_{b}_{jt}")
                nc.vector.tensor_scalar(
                    out=nbias[:], in0=cc[:], scalar1=rl[:], scalar2=-1.0,
                    op0=ALU.mult, op1=ALU.mult,
                )
                lo_dst, lo_src = p2[:, jt, 0:512], ulo[jt][:]
                hi_dst = p2[:, jt, 512:768]
                hi_src = (uhiA[:, jt * 256:(jt + 1) * 256] if jt < 2
                          else uhiB[:, 0:256])
                if jt % 2 == 0:
                    act_pairs, dve_pairs = [(lo_dst, lo_src)], \
                        [(hi_dst, hi_src)]
                else:
                    act_pairs, dve_pairs = [(hi_dst, hi_src)], \
                        [(lo_dst, lo_src)]
                for dst, src in act_pairs:
                    nc.scalar.activation(dst, src, AF.Identity,
                                         bias=nbias[:], scale=rl[:])
                for dst, src in dve_pairs:
                    nc.vector.tensor_scalar(
                        out=dst, in0=src,
                        scalar1=cc[:], scalar2=rl[:],
                        op0=ALU.subtract, op1=ALU.mult,
                    )
            ep_state[b] = p2

        def ep2(b):
            p2 = ep_state.pop(b)
            p2T = epi.tile([128, ET, J], F16, tag="p2T", name=f"p2T_{b}")
            if b < B_LOC - 1:
                for jt in range(JT):
                    nc.sync.dma_start_transpose(
                        p2T[:, :, jt * 128:(jt + 1) * 128], p2[:, jt, :]
                    )
            else:
                for et in range(ET):
                    tp = pt.tile([128, 384], F16, tag="tp",
                                 name=f"ep_tp_{b}_{et}")
                    for jt in range(JT):
                        nc.tensor.transpose(
                            tp[:, jt * 128:(jt + 1) * 128],
                            p2[:, jt, et * 128:(et + 1) * 128],
                            id_sb[:],
                        )
                    if et % 2 == 0:
                        nc.scalar.copy(p2T[:, et, :], tp[:])
                    else:
                        nc.vector.tensor_copy(p2T[:, et, :], tp[:])
            ep_state[b] = p2T

        def ep3(b):
            p2T = ep_state.pop(b)
            ctxT = epi.tile([128, ET, NQ], F16, tag="ctxT", name=f"ctxT_{b}")
            for g in range(3):
                cp = pt.tile([128, 2, NQ], F32, tag="tp", name=f"cp_{b}_g{g}")
                for hh in range(4):
                    h = g * 4 + hh
                    dst = cp[(hh % 2) * 64:(hh % 2) * 64 + 64, hh // 2, :]
                    for et in range(ET):
                        nc.tensor.matmul(
                            dst,
                            wv_sb[:, et, h * 64:(h + 1) * 64],
                            p2T[:, et, h * NQ:(h + 1) * NQ],
                            start=(et == 0 and hh <= 1), stop=(et == ET - 1),
                            skip_group_check=True,
                        )
                if g % 2 == 0:
                    nc.scalar.copy(ctxT[:, 2 * g:2 * g + 2, :], cp[:])
                else:
                    nc.vector.tensor_copy(ctxT[:, 2 * g:2 * g + 2, :], cp[:])

            oc = epi.tile([128, ET, NQ], F32, tag="oc", name=f"oc_{b}")
            for g in range(2):
                po = pt.tile([128, 3, NQ], F32, tag="tp", name=f"po_{b}_{g}")
                for dd in range(3):
                    dt = g * 3 + dd
                    for g2 in range(ET):
                        nc.tensor.matmul(
                            po[:, dd, :],
                            wo_sb[:, g2, dt * 128:(dt + 1) * 128],
                            ctxT[:, g2, :],
                            start=(g2 == 0 and dd == 0), stop=(g2 == ET - 1),
                            skip_group_check=True,
                        )
                if g == 0:
                    nc.scalar.copy(oc[:, 0:3, :], po[:])
                else:
                    nc.vector.tensor_copy(oc[:, 3:6, :], po[:])
                nc.sync.dma_start(y_d[b, :, 3 * g:3 * (g + 1), :],
                                  oc[:, 3 * g:3 * g + 3, :])

        NB_B = NB // B_LOC
        for bi in range(NB + 4):
            if bi < NB:
                stage_a(bi)
            if 1 <= bi < NB + 1:
                stage_st(bi - 1)
            if 2 <= bi < NB + 2:
                stage_b(bi - 2)
            for b in range(B_LOC):
                fin = (b + 1) * NB_B + 1
                if bi == fin:
                    ep1(b)
                elif bi == fin + 1:
                    ep2(b)
                elif bi == fin + 2:
                    ep3(b)

    nc.compile()
    return nc


_NC_CACHE = None


def _get_program():
    global _NC_CACHE
    if _NC_CACHE is None:
        _NC_CACHE = _build_program()
    return _NC_CACHE


def _fold_weights(queries, Wq, Wkv, Wo, gamma, beta):
    q = queries.astype(np.float64) @ Wq.astype(np.float64)
    qh = q.reshape(NQ, H, DH)
    Wk = Wkv[:, :INNER].astype(np.float64)
    Wv = Wkv[:, INNER:].astype(np.float64)
    Wk_h = Wk.reshape(DIM, H, DH)
    qt = np.einsum("nhd,ehd->hne", qh, Wk_h, optimize=True).reshape(J, DIM)
    A = (gamma.astype(np.float64)[:, None] * qt.T) / (DH ** 0.5)
    Ac = A - A.mean(axis=0, keepdims=True)
    Wvp = gamma.astype(np.float64)[:, None] * Wv
    bvwo = (beta.astype(np.float64) @ Wv) @ Wo.astype(np.float64)

    acs = Ac * float(2 ** ACLOG2)
    ac8 = acs.astype(np.float32).astype(E4NP)
    dac = (acs - ac8.astype(np.float64)).astype(np.float32).astype(E4NP)

    def dr_pack(m8):
        return np.ascontiguousarray(
            m8.reshape(KB, 2, 128, J).transpose(2, 0, 1, 3)
        )

    def tile6(m):
        return np.ascontiguousarray(
            m.reshape(ET, 128, -1).transpose(1, 0, 2)
        ).astype(np.float16)

    return (
        dr_pack(ac8),
        dr_pack(dac),
        tile6(Wvp),
        tile6(Wo.astype(np.float64)),
        bvwo.astype(np.float32),
    )


def kernel(encoder_outputs, queries, Wq, Wkv, Wo, ln_gamma, ln_beta):
    enc = np.asarray(encoder_outputs, dtype=np.float32)
    x8 = enc.astype(E4NP)
    r8 = (enc - x8.astype(np.float32)).astype(E4NP)
    xt = np.ascontiguousarray(
        x8.reshape(B, N_CHUNKS, 128, KB, 2, 128).transpose(0, 1, 5, 3, 4, 2)
    ).reshape(B, N_CHUNKS, 128, KB * 2 * 128)
    queries = np.asarray(queries, dtype=np.float32)
    Wq = np.asarray(Wq, dtype=np.float32)
    Wkv = np.asarray(Wkv, dtype=np.float32)
    Wo_np = np.asarray(Wo, dtype=np.float32)
    gamma = np.asarray(ln_gamma, dtype=np.float32)
    beta = np.asarray(ln_beta, dtype=np.float32)

    ac8, dac8, wv_t, wo_t, bvwo = _fold_weights(
        queries, Wq, Wkv, Wo_np, gamma, beta
    )

    nc = _get_program()
    in_maps = [
        {
            "x8": x8[c * B_LOC:(c + 1) * B_LOC],
            "r8": r8[c * B_LOC:(c + 1) * B_LOC],
            "xt": xt[c * B_LOC:(c + 1) * B_LOC],
            "ac": ac8,
            "dac": dac8,
            "wv": wv_t,
            "wo": wo_t,
            "ident": np.eye(128, dtype=np.float16),
        }
        for c in range(N_CORES)
    ]
    res = run_bass_kernel_spmd(nc, in_maps, list(range(N_CORES)))
    y = np.concatenate([res.results[c]["y"] for c in range(N_CORES)], axis=0)
    y = y.reshape(B, 128, ET, NQ).transpose(0, 3, 2, 1).reshape(B, NQ, DIM)
    return np.ascontiguousarray(y + bvwo[None, None, :]).astype(np.float32)
